# revision 6
# baseline (speedup 1.0000x reference)
"""EMA recurrence kernel for Trainium2 (8 NeuronCores, batch-parallel).

Computes c[b,t,d] = x[b,t,d] + decay * c[b,t-1,d]  (decay = sigmoid(decay_logit))
for x of shape (8, 4096, 2048) fp32, as a blocked scan:

  - T is split into chunks of L=127 rows. Within a chunk the scan is a
    triangular matmul: out[t,d] = sum_{s<=t} decay^(t-s) x[s,d].
  - The cross-chunk carry (c at the last row of the previous chunk) is folded
    into the same matmul as an extra contraction row whose weight column is
    decay^(t+1) — so each chunk is ONE fp32 matmul per 512-wide D tile.
  - Layout: the carry input row lives at SBUF partition 0 (x rows at
    partitions 1..127), and the matmul's output columns are permuted so that
    PSUM partition 0 holds the chunk's LAST scan position (the next carry)
    and partitions 1..127 hold scan positions 0..126.  All compute-engine
    access patterns therefore start at partition 0 (the BIR verifier rejects
    engine APs starting at non-32-aligned partitions); only DMA (which has
    no partition-alignment restriction) touches rows 1..127.  Chunk 0 has no
    carry: it uses its own weight matrix with x rows at partitions 0..126.
  - All PSUM reads and the carry chain run on VectorE only, so each matmul
    needs at most 2 semaphore waits (DVE + DMA) — the LDWEIGHTS side of a
    matmul has only 2 hardware wait slots.
  - Batch b is sharded across the 8 cores (one b per core); each core streams
    its [4096, 2048] slice through SBUF in contiguous ~1MB chunks.
"""

import os
import sys

os.environ.setdefault("MYCRO_LOCAL_CACHE", "1")
if "/opt/trn_rl_repo" not in sys.path:
    sys.path.insert(0, "/opt/trn_rl_repo")

from contextlib import ExitStack

import numpy as np

B, T, D = 8, 4096, 2048
L = 127                 # x rows per main chunk (+1 carry row = K of 128)
NCHUNK = T // L         # 32 full chunks
TAIL = T - NCHUNK * L   # 32 trailing rows
DT = 512                # D tile width (one PSUM bank of fp32)
NT = D // DT            # 4 D tiles
N_CORES = 8
LTW = 128 + 128 + (TAIL + 1)  # packed weight tensor width

_compiled = {}


def _build_weights(decay_logit: np.ndarray):
    # Match the reference: decay = sigmoid(decay_logit) evaluated in fp32,
    # powers computed in fp64 from that fp32 value, rounded to fp32.
    logit = np.float64(np.asarray(decay_logit, dtype=np.float32))
    decay = np.float64(np.float32(1.0 / (1.0 + np.exp(-logit))))

    def lhs_t(rows, with_carry):
        # lhsT is [K, M]; out = lhsT.T @ rhs.
        # Output column m: m=0 is the carry-out (scan position rows-1),
        # m=1+t is scan position t.
        # Contraction p: with_carry -> p=0 is the carry row, p=1+s is x row s;
        # else p=s is x row s.
        pw = decay ** np.arange(rows + 1, dtype=np.float64)
        tri = np.zeros((rows, rows), np.float64)
        for s in range(rows):
            tri[s, s:] = pw[: rows - s]
        k = rows + 1 if with_carry else rows
        m = np.zeros((k, rows + 1), np.float64)
        if with_carry:
            m[0, 0] = pw[rows]          # carry -> carry-out
            m[1:, 0] = pw[rows - 1 :: -1]
            m[0, 1:] = pw[1:]           # carry -> position t
            m[1:, 1:] = tri
        else:
            m[:, 0] = pw[rows - 1 :: -1]
            m[:, 1:] = tri
        return m.astype(np.float32)

    lt_first = lhs_t(L, with_carry=False)   # [127, 128]
    lt_main = lhs_t(L, with_carry=True)     # [128, 128]
    lt_tail = lhs_t(TAIL, with_carry=True)  # [33, 33]

    packed = np.zeros((128, LTW), np.float32)
    packed[:127, 0:128] = lt_first
    packed[:, 128:256] = lt_main
    packed[: TAIL + 1, 256 : 256 + TAIL + 1] = lt_tail
    return packed


def _build_program():
    import concourse.bacc as bacc
    import concourse.mybir as mybir
    from concourse.tile import TileContext

    f32 = mybir.dt.float32
    nc = bacc.Bacc(trn_type="TRN2", target_bir_lowering=False, debug=False)

    x_d = nc.dram_tensor("x", [T, D], f32, kind="ExternalInput")
    lt_d = nc.dram_tensor("lt_all", [128, LTW], f32, kind="ExternalInput")
    y_d = nc.dram_tensor("y", [T, D], f32, kind="ExternalOutput")

    with TileContext(nc) as tc, ExitStack() as ctx:
        const = ctx.enter_context(tc.tile_pool(name="const", bufs=1))
        lt = const.tile([128, LTW], f32, name="lt")
        nc.sync.dma_start(lt[:, :], lt_d[:, :])
        lt_first = lt[0:L, 0:128]
        lt_main = lt[0:128, 128:256]
        lt_tail = lt[0 : TAIL + 1, 256 : 256 + TAIL + 1]

        xin_pool = ctx.enter_context(tc.tile_pool(name="xin", bufs=4))
        yout_pool = ctx.enter_context(tc.tile_pool(name="yout", bufs=4))
        ps_pool = ctx.enter_context(tc.tile_pool(name="ps", bufs=8, space="PSUM"))

        # Chunk input tiles. Chunk 0: x rows at partitions 0..126 (no carry).
        # Chunks 1..: partition 0 = carry row, partitions 1..rows = x rows.
        chunk_rows = [L] * NCHUNK + [TAIL]
        xts = []
        for k, rows in enumerate(chunk_rows):
            if k == 0:
                xt = xin_pool.tile([L, D], f32, name="xt0", tag="xin")
                nc.sync.dma_start(xt[:, :], x_d[0:L, :])
            else:
                xt = xin_pool.tile([rows + 1, D], f32, name=f"xt{k}", tag="xin")
                nc.sync.dma_start(xt[1 : rows + 1, :], x_d[k * L : k * L + rows, :])
            xts.append(xt)

        for k, rows in enumerate(chunk_rows):
            lhsT = lt_first if k == 0 else (lt_tail if rows == TAIL else lt_main)
            xt = xts[k]
            yt = yout_pool.tile([rows + 1, D], f32, name=f"yt{k}", tag="yout")
            for j in range(NT):
                ps = ps_pool.tile([rows + 1, DT], f32, name=f"ps{k}_{j}", tag="ps")
                nc.tensor.matmul(
                    ps[:, :],
                    lhsT,
                    xt[:, j * DT : (j + 1) * DT],
                    start=True,
                    stop=True,
                )
                nc.vector.tensor_copy(yt[:, j * DT : (j + 1) * DT], ps[:, :])
            if k + 1 < len(chunk_rows):
                # carry row for next chunk = yt row 0 (scan position rows-1);
                # SBUF->SBUF fp32 copy runs in the DVE 2x mode
                nc.vector.tensor_copy(xts[k + 1][0:1, :], yt[0:1, :])
            # rows 1..rows of yt hold scan positions 0..rows-1; output DMA on
            # the second HWDGE ring (scalar) so in/out streams ride separate rings
            nc.scalar.dma_start(y_d[k * L : k * L + rows, :], yt[1 : rows + 1, :])

    nc.finalize()
    return nc


def _get_program():
    if "nc" not in _compiled:
        _compiled["nc"] = _build_program()
    return _compiled["nc"]


def _install_profile_hook():
    """The container's `antenv` lacks `axon_hooks`, so NTFF profiling under
    axon degrades silently. Synthesize the module and install the ctypes hook
    from trn_agent_boot (same thing boot() would have done)."""
    if "antenv.axon_hooks" in sys.modules:
        return
    import types

    import antenv

    mod = types.ModuleType("antenv.axon_hooks")
    state = {"hook": None}
    mod.set_axon_ntff_profile_hook = lambda h: state.__setitem__("hook", h)
    mod.get_axon_ntff_profile_hook = lambda: state["hook"]
    sys.modules["antenv.axon_hooks"] = mod
    antenv.axon_hooks = mod

    from trn_agent_boot.trn_boot import _ntff_profile_via_ctypes

    mod.set_axon_ntff_profile_hook(
        _ntff_profile_via_ctypes("/opt/axon/libaxon_pjrt.so")
    )

    # no S3 in this container — keep artifacts local
    from concourse import bass_utils

    bass_utils.upload_artifacts = lambda tmpdir: tmpdir


def _run(x, decay_logit, trace=False):
    from concourse.bass_utils import run_bass_kernel_spmd

    if trace:
        _install_profile_hook()

    x = np.ascontiguousarray(np.asarray(x, dtype=np.float32))
    assert x.shape == (B, T, D), x.shape
    lt_all = _build_weights(decay_logit)

    nc = _get_program()
    in_maps = [
        {"x": np.ascontiguousarray(x[b]), "lt_all": lt_all} for b in range(N_CORES)
    ]
    res = run_bass_kernel_spmd(
        nc,
        in_maps,
        core_ids=list(range(N_CORES)),
        trace=trace,
        trace_cores=[0] if trace else None,
    )
    y = np.stack([res.results[b]["y"] for b in range(N_CORES)], axis=0)
    return y, res


def kernel(x, decay_logit):
    y, _ = _run(x, decay_logit, trace=False)
    return y


def kernel_traced(x, decay_logit):
    """Like kernel() but returns (y, BassKernelResults) with NTFF profile."""
    return _run(x, decay_logit, trace=True)


# revision 7
# speedup vs baseline: 3.8373x; 3.8373x over previous
"""EMA recurrence kernel for Trainium2 (8 NeuronCores, batch-parallel).

Computes c[b,t,d] = x[b,t,d] + decay * c[b,t-1,d]  (decay = sigmoid(decay_logit))
for x of shape (8, 4096, 2048) fp32, as a blocked scan:

  - T is split into chunks of L=127 rows. Within a chunk the scan is a
    triangular matmul: out[t,d] = sum_{s<=t} decay^(t-s) x[s,d].
  - The cross-chunk carry (c at the last row of the previous chunk) is folded
    into the same matmul as an extra contraction row whose weight column is
    decay^(t+1) — so each chunk is ONE fp32 matmul per 512-wide D tile.
  - Layout: the carry input row lives at SBUF partition 0 (x rows at
    partitions 1..127), and the matmul's output columns are permuted so that
    PSUM partition 0 holds the chunk's LAST scan position (the next carry)
    and partitions 1..127 hold scan positions 0..126.  All compute-engine
    access patterns therefore start at partition 0 (the BIR verifier rejects
    engine APs starting at non-32-aligned partitions); only DMA (which has
    no partition-alignment restriction) touches rows 1..127.  Chunk 0 has no
    carry: it uses its own weight matrix with x rows at partitions 0..126.
  - All PSUM reads and the carry chain run on VectorE only, so each matmul
    needs at most 2 semaphore waits (DVE + DMA) — the LDWEIGHTS side of a
    matmul has only 2 hardware wait slots.
  - Batch b is sharded across the 8 cores (one b per core); each core streams
    its [4096, 2048] slice through SBUF in contiguous ~1MB chunks.
"""

import os
import sys

os.environ.setdefault("MYCRO_LOCAL_CACHE", "1")
if "/opt/trn_rl_repo" not in sys.path:
    sys.path.insert(0, "/opt/trn_rl_repo")

from contextlib import ExitStack

import numpy as np

B, T, D = 8, 4096, 2048
L = 127                 # x rows per main chunk (+1 carry row = K of 128)
NCHUNK = T // L         # 32 full chunks
TAIL = T - NCHUNK * L   # 32 trailing rows
DT = 512                # D tile width (one PSUM bank of fp32)
NT = D // DT            # 4 D tiles
N_CORES = 8
LTW = 128 + 128 + (TAIL + 1)  # packed weight tensor width

_compiled = {}


def _build_weights(decay_logit: np.ndarray):
    # Match the reference: decay = sigmoid(decay_logit) evaluated in fp32,
    # powers computed in fp64 from that fp32 value, rounded to fp32.
    logit = np.float64(np.asarray(decay_logit, dtype=np.float32))
    decay = np.float64(np.float32(1.0 / (1.0 + np.exp(-logit))))

    def lhs_t(rows, with_carry):
        # lhsT is [K, M]; out = lhsT.T @ rhs.
        # Output column m: m=0 is the carry-out (scan position rows-1),
        # m=1+t is scan position t.
        # Contraction p: with_carry -> p=0 is the carry row, p=1+s is x row s;
        # else p=s is x row s.
        pw = decay ** np.arange(rows + 1, dtype=np.float64)
        tri = np.zeros((rows, rows), np.float64)
        for s in range(rows):
            tri[s, s:] = pw[: rows - s]
        k = rows + 1 if with_carry else rows
        m = np.zeros((k, rows + 1), np.float64)
        if with_carry:
            m[0, 0] = pw[rows]          # carry -> carry-out
            m[1:, 0] = pw[rows - 1 :: -1]
            m[0, 1:] = pw[1:]           # carry -> position t
            m[1:, 1:] = tri
        else:
            m[:, 0] = pw[rows - 1 :: -1]
            m[:, 1:] = tri
        return m.astype(np.float32)

    lt_first = lhs_t(L, with_carry=False)   # [127, 128]
    lt_main = lhs_t(L, with_carry=True)     # [128, 128]
    lt_tail = lhs_t(TAIL, with_carry=True)  # [33, 33]

    packed = np.zeros((128, LTW), np.float32)
    packed[:127, 0:128] = lt_first
    packed[:, 128:256] = lt_main
    packed[: TAIL + 1, 256 : 256 + TAIL + 1] = lt_tail
    return packed


def _build_program():
    import concourse.bacc as bacc
    import concourse.mybir as mybir
    from concourse.tile import TileContext

    f32 = mybir.dt.float32
    nc = bacc.Bacc(trn_type="TRN2", target_bir_lowering=False, debug=False)

    x_d = nc.dram_tensor("x", [T, D], f32, kind="ExternalInput")
    lt_d = nc.dram_tensor("lt_all", [128, LTW], f32, kind="ExternalInput")
    y_d = nc.dram_tensor("y", [T, D], f32, kind="ExternalOutput")

    with TileContext(nc) as tc, ExitStack() as ctx:
        const = ctx.enter_context(tc.tile_pool(name="const", bufs=1))
        lt = const.tile([128, LTW], f32, name="lt")
        nc.sync.dma_start(lt[:, :], lt_d[:, :])
        lt_first = lt[0:L, 0:128]
        lt_main = lt[0:128, 128:256]
        lt_tail = lt[0 : TAIL + 1, 256 : 256 + TAIL + 1]

        xin_pool = ctx.enter_context(tc.tile_pool(name="xin", bufs=4))
        yout_pool = ctx.enter_context(tc.tile_pool(name="yout", bufs=4))
        ps_pool = ctx.enter_context(tc.tile_pool(name="ps", bufs=8, space="PSUM"))

        # Chunk input tiles. Chunk 0: x rows at partitions 0..126 (no carry).
        # Chunks 1..: partition 0 = carry row, partitions 1..rows = x rows.
        chunk_rows = [L] * NCHUNK + [TAIL]
        xts = []
        for k, rows in enumerate(chunk_rows):
            if k == 0:
                xt = xin_pool.tile([L, D], f32, name="xt0", tag="xin")
                nc.gpsimd.dma_start(xt[:, :], x_d[0:L, :])
            else:
                xt = xin_pool.tile([rows + 1, D], f32, name=f"xt{k}", tag="xin")
                nc.gpsimd.dma_start(xt[1 : rows + 1, :], x_d[k * L : k * L + rows, :])
            xts.append(xt)

        for k, rows in enumerate(chunk_rows):
            lhsT = lt_first if k == 0 else (lt_tail if rows == TAIL else lt_main)
            xt = xts[k]
            yt = yout_pool.tile([rows + 1, D], f32, name=f"yt{k}", tag="yout")
            for j in range(NT):
                ps = ps_pool.tile([rows + 1, DT], f32, name=f"ps{k}_{j}", tag="ps")
                nc.tensor.matmul(
                    ps[:, :],
                    lhsT,
                    xt[:, j * DT : (j + 1) * DT],
                    start=True,
                    stop=True,
                )
                nc.vector.tensor_copy(yt[:, j * DT : (j + 1) * DT], ps[:, :])
            if k + 1 < len(chunk_rows):
                # carry row for next chunk = yt row 0 (scan position rows-1);
                # SBUF->SBUF fp32 copy runs in the DVE 2x mode
                nc.vector.tensor_copy(xts[k + 1][0:1, :], yt[0:1, :])
            # rows 1..rows of yt hold scan positions 0..rows-1; output DMA on
            # the second HWDGE ring (scalar) so in/out streams ride separate rings
            nc.gpsimd.dma_start(y_d[k * L : k * L + rows, :], yt[1 : rows + 1, :])

    nc.finalize()
    return nc


def _get_program():
    if "nc" not in _compiled:
        _compiled["nc"] = _build_program()
    return _compiled["nc"]


def _install_profile_hook():
    """The container's `antenv` lacks `axon_hooks`, so NTFF profiling under
    axon degrades silently. Synthesize the module and install the ctypes hook
    from trn_agent_boot (same thing boot() would have done)."""
    if "antenv.axon_hooks" in sys.modules:
        return
    import types

    import antenv

    mod = types.ModuleType("antenv.axon_hooks")
    state = {"hook": None}
    mod.set_axon_ntff_profile_hook = lambda h: state.__setitem__("hook", h)
    mod.get_axon_ntff_profile_hook = lambda: state["hook"]
    sys.modules["antenv.axon_hooks"] = mod
    antenv.axon_hooks = mod

    from trn_agent_boot.trn_boot import _ntff_profile_via_ctypes

    mod.set_axon_ntff_profile_hook(
        _ntff_profile_via_ctypes("/opt/axon/libaxon_pjrt.so")
    )

    # no S3 in this container — keep artifacts local
    from concourse import bass_utils

    bass_utils.upload_artifacts = lambda tmpdir: tmpdir


def _run(x, decay_logit, trace=False):
    from concourse.bass_utils import run_bass_kernel_spmd

    if trace:
        _install_profile_hook()

    x = np.ascontiguousarray(np.asarray(x, dtype=np.float32))
    assert x.shape == (B, T, D), x.shape
    lt_all = _build_weights(decay_logit)

    nc = _get_program()
    in_maps = [
        {"x": np.ascontiguousarray(x[b]), "lt_all": lt_all} for b in range(N_CORES)
    ]
    res = run_bass_kernel_spmd(
        nc,
        in_maps,
        core_ids=list(range(N_CORES)),
        trace=trace,
        trace_cores=[0] if trace else None,
    )
    y = np.stack([res.results[b]["y"] for b in range(N_CORES)], axis=0)
    return y, res


def kernel(x, decay_logit):
    y, _ = _run(x, decay_logit, trace=False)
    return y


def kernel_traced(x, decay_logit):
    """Like kernel() but returns (y, BassKernelResults) with NTFF profile."""
    return _run(x, decay_logit, trace=True)
